# revision 40
# baseline (speedup 1.0000x reference)
"""v12: v10 + fp16 output stores (host upcasts) — halves output write traffic.

Identical to the v1 run: per-(gate,m) weight tiles used for both 512-row
chunks, i-first gate order in phase B, ctb-last o-gate accumulation,
whole-tensor activation loads (xbl split 2/6 on gpsimd, xbr on scalar).
"""

import sys

if "/opt/trn_rl_repo" not in sys.path:
    sys.path.insert(0, "/opt/trn_rl_repo")

import numpy as np

N_CORES = 8
N = 8192
D = 1024
P = 128
NL = N // N_CORES
KB = D // P
MT = D // P
R = 512
NCH = NL // R

_GATES_B = [
    ("i",  40, 1, "sig",  [("hbl", 0), ("hbr", 8), ("xtb", 32),
                           ("cbl", 16), ("cbr", 24)], None),
    ("xf", 8,  None, None, [("xtb", 0)], None),
    ("fl", 32, 2, "sig",  [("hbl", 0), ("hbr", 8), ("cbl", 16), ("cbr", 24)], "xf"),
    ("fr", 32, 3, "sig",  [("hbl", 0), ("hbr", 8), ("cbl", 16), ("cbr", 24)], "xf"),
    ("g",  24, 4, "tanh", [("hbl", 0), ("hbr", 8), ("xtb", 16)], None),
]

_compiled = {}


def _build():
    import concourse.mybir as mybir
    import concourse.tile as tile
    from concourse import bacc

    F32 = mybir.dt.float32
    F16 = mybir.dt.float16
    SIG = mybir.ActivationFunctionType.Sigmoid
    TANH = mybir.ActivationFunctionType.Tanh

    nc = bacc.Bacc("TRN2", target_bir_lowering=False, debug=False)

    def din(name, shape, dt):
        return nc.dram_tensor(name, shape, dt, kind="ExternalInput").ap()

    def dout(name, shape, dt):
        return nc.dram_tensor(name, shape, dt, kind="ExternalOutput").ap()

    xb_l = din("xb_l", [D, NL], F16)
    xb_r = din("xb_r", [D, NL], F16)
    hb_l = din("hb_l", [D, NL], F16)
    hb_r = din("hb_r", [D, NL], F16)
    cb_l = din("cb_l", [D, NL], F16)
    cb_r = din("cb_r", [D, NL], F16)
    wz = din("wz", [MT, P, 16, P], F16)
    wi = din("wi", [MT, P, 40, P], F16)
    wfl = din("wfl", [MT, P, 32, P], F16)
    wfr = din("wfr", [MT, P, 32, P], F16)
    wg = din("wg", [MT, P, 24, P], F16)
    wo = din("wo", [MT, P, 32, P], F16)
    wxf = din("wxf", [MT, P, 8, P], F16)
    wmap = {"i": wi, "xf": wxf, "fl": wfl, "fr": wfr, "g": wg}
    bias = din("bias", [P, 6, MT], F32)

    xT_o = dout("xT_o", [D, NL], F16)
    hT_o = dout("hT_o", [D, NL], F16)
    cT_o = dout("cT_o", [D, NL], F16)

    def r3(ap):
        return ap.rearrange("(k p) n -> p k n", p=P)

    with tile.TileContext(nc) as tc:
        with (
            tc.tile_pool(name="acts", bufs=1) as apool,
            tc.tile_pool(name="w", bufs=3) as wpool,
            tc.tile_pool(name="gates", bufs=8) as gpool,
            tc.tile_pool(name="work", bufs=10) as wkpool,
            tc.tile_pool(name="ps", bufs=8, space="PSUM") as pspool,
            tc.tile_pool(name="cst", bufs=1) as cpool,
        ):
            bias_t = cpool.tile([P, 6, MT], F32, name="bias_t")
            nc.sync.dma_start(bias_t[:], bias[:])

            xbl_t = apool.tile([P, KB, NL], F16, tag="xbl", name="xbl")
            nc.gpsimd.dma_start(xbl_t[:, :2, :], r3(xb_l)[:, :2, :])
            nc.gpsimd.dma_start(xbl_t[:, 2:, :], r3(xb_l)[:, 2:, :])
            xbr_t = apool.tile([P, KB, NL], F16, tag="xbr", name="xbr")
            nc.scalar.dma_start(xbr_t[:, :2, :], r3(xb_r)[:, :2, :])
            nc.scalar.dma_start(xbr_t[:, 2:, :], r3(xb_r)[:, 2:, :])

            def lda(name, dram):
                t = apool.tile([P, KB, NL], F16, tag=name, name=name)
                nc.gpsimd.dma_start(t[:], r3(dram)[:])
                return t

            hbl_t = lda("hbl", hb_l)
            hbr_t = lda("hbr", hb_r)
            cbl_t = lda("cbl", cb_l)
            cbr_t = lda("cbr", cb_r)

            xtb_t = apool.tile([P, KB, NL], F16, tag="xtb", name="xtb")
            ctb_t = apool.tile([P, KB, NL], F16, tag="ctb", name="ctb")
            parts = {"hbl": hbl_t, "hbr": hbr_t, "cbl": cbl_t,
                     "cbr": cbr_t, "xtb": xtb_t, "ctb": ctb_t}

            def chunks():
                return [slice(c * R, (c + 1) * R) for c in range(NCH)]

            for m in range(MT):
                w_t = wpool.tile([P, 16, P], F16, tag="w", name="wz_t")
                nc.sync.dma_start(w_t[:], wz[m])
                for cs in chunks():
                    ps = pspool.tile([P, R], F32, tag="ps", name="ps_z")
                    for kt in range(16):
                        rhs = (xbl_t if kt < KB else xbr_t)[:, kt % KB, cs]
                        nc.tensor.matmul(ps[:], w_t[:, kt, :], rhs,
                                         start=(kt == 0), stop=(kt == 15))
                    z_t = wkpool.tile([P, R], F32, tag="wk", name="z_t")
                    nc.scalar.activation(z_t[:], ps[:], SIG,
                                         bias=bias_t[:, 0, m, None])
                    d_t = wkpool.tile([P, R], F32, tag="wk", name="d_t")
                    nc.vector.tensor_sub(d_t[:], xbl_t[:, m, cs],
                                         xbr_t[:, m, cs])
                    xrf_m = wkpool.tile([P, R], F32, tag="wk", name="xrf_m")
                    nc.vector.tensor_copy(xrf_m[:], xbr_t[:, m, cs])
                    nc.vector.tensor_mul(d_t[:], d_t[:], z_t[:])
                    xt_m = wkpool.tile([P, R], F16, tag="wk", name="xt_m")
                    nc.vector.tensor_add(xt_m[:], d_t[:], xrf_m[:])
                    nc.scalar.dma_start(r3(xT_o)[:, m, cs], xt_m[:])
                    nc.vector.tensor_copy(xtb_t[:, m, cs], xt_m[:])

            for m in range(MT):
                gt = {}
                xfp = []
                for (gname, Kt, b_idx, fn, rparts, xkey) in _GATES_B:
                    w_t = wpool.tile([P, Kt, P], F16, tag="w",
                                     name=f"w_{gname}")
                    nc.sync.dma_start(w_t[:], wmap[gname][m])
                    per_chunk = []
                    for cs in chunks():
                        ps = pspool.tile([P, R], F32, tag="ps",
                                         name=f"ps_{gname}")
                        n_done = 0
                        for (pname, koff) in rparts:
                            pt = parts[pname]
                            for j in range(KB):
                                nc.tensor.matmul(
                                    ps[:], w_t[:, koff + j, :], pt[:, j, cs],
                                    start=(n_done == 0),
                                    stop=(n_done == Kt - 1))
                                n_done += 1
                        if gname == "xf":
                            xf_c = gpool.tile([P, R], F32, tag="gate",
                                              name="xfp")
                            nc.scalar.copy(xf_c[:], ps[:])
                            xfp.append(xf_c)
                            continue
                        if xkey == "xf":
                            nc.vector.tensor_add(ps[:], ps[:],
                                                 xfp[len(per_chunk)][:])
                        g_t = gpool.tile([P, R], F32, tag="gate",
                                         name=f"g_{gname}")
                        nc.scalar.activation(
                            g_t[:], ps[:], SIG if fn == "sig" else TANH,
                            bias=bias_t[:, b_idx, m, None])
                        per_chunk.append(g_t)
                    if gname != "xf":
                        gt[gname] = per_chunk
                for ci, cs in enumerate(chunks()):
                    cfl_m = wkpool.tile([P, R], F32, tag="wk", name="cfl_m")
                    nc.vector.tensor_copy(cfl_m[:], cbl_t[:, m, cs])
                    cfr_m = wkpool.tile([P, R], F32, tag="wk", name="cfr_m")
                    nc.vector.tensor_copy(cfr_m[:], cbr_t[:, m, cs])
                    ct_m = wkpool.tile([P, R], F32, tag="wk", name="ct_m")
                    nc.vector.tensor_mul(ct_m[:], gt["fl"][ci][:], cfl_m[:])
                    t2 = wkpool.tile([P, R], F32, tag="wk", name="t2")
                    nc.vector.tensor_mul(t2[:], gt["fr"][ci][:], cfr_m[:])
                    nc.vector.tensor_add(ct_m[:], ct_m[:], t2[:])
                    nc.vector.tensor_mul(t2[:], gt["i"][ci][:], gt["g"][ci][:])
                    nc.vector.tensor_add(ct_m[:], ct_m[:], t2[:])
                    cts = wkpool.tile([P, R], F16, tag="wk", name="cts")
                    nc.vector.tensor_copy(cts[:], ct_m[:])
                    nc.scalar.dma_start(r3(cT_o)[:, m, cs], cts[:])
                    nc.vector.tensor_copy(ctb_t[:, m, cs], ct_m[:])

            o_parts = [("hbl", 0), ("hbr", 8), ("xtb", 24), ("ctb", 16)]
            tct7 = cpool.tile([P, NCH, R], F32, name="tct7")
            for m in range(MT):
                w_t = wpool.tile([P, 32, P], F16, tag="w", name="wo_t")
                nc.sync.dma_start(w_t[:], wo[m])
                last = (m == MT - 1)
                for ci, cs in enumerate(chunks()):
                    # The very last chunk runs in 256-col halves so its
                    # activation/store chain overlaps its own matmuls.
                    halves = ([slice(0, 256), slice(256, 512)]
                              if (last and ci == NCH - 1) else [slice(0, R)])
                    for hs in halves:
                        rh = hs.stop - hs.start
                        rs = slice(cs.start + hs.start, cs.start + hs.stop)
                        ps = pspool.tile([P, rh], F32, tag="ps", name="ps_o")
                        kt = 0
                        for pname, koff in o_parts:
                            pt = parts[pname]
                            for j in range(KB):
                                nc.tensor.matmul(ps[:], w_t[:, koff + j, :],
                                                 pt[:, j, rs],
                                                 start=(kt == 0),
                                                 stop=(kt == 31))
                                kt += 1
                        o_t = wkpool.tile([P, rh], F32, tag="wk", name="o_t")
                        nc.scalar.activation(o_t[:], ps[:], SIG,
                                             bias=bias_t[:, 5, m, None])
                        if last:
                            tct_v = tct7[:, ci, hs]
                        else:
                            tct_m = wkpool.tile([P, rh], F32, tag="wk",
                                                name="tct_m")
                            nc.scalar.activation(tct_m[:], ctb_t[:, m, rs],
                                                 TANH)
                            tct_v = tct_m[:]
                        ht_m = wkpool.tile([P, rh], F16, tag="wk",
                                           name="ht_m")
                        nc.vector.tensor_mul(ht_m[:], o_t[:], tct_v)
                        nc.scalar.dma_start(r3(hT_o)[:, m, rs], ht_m[:])
                if m == 0:
                    # Precompute the last m-tile's tanh(c) off the tail.
                    for ci, cs in enumerate(chunks()):
                        nc.scalar.activation(tct7[:, ci, :],
                                             ctb_t[:, MT - 1, cs], TANH)

    nc.compile()
    return nc


def _get_compiled():
    if "k" not in _compiled:
        _compiled["k"] = _build()
    return _compiled["k"]


def _prep_weight(w_km):
    K = w_km.shape[0]
    kt = K // P
    w = w_km.reshape(kt, P, MT, P)
    w = np.ascontiguousarray(w.transpose(2, 1, 0, 3))
    return w.astype(np.float16)


def _host_prep(inp):
    f32 = np.float32
    t = {k: np.asarray(inp[k], dtype=f32).T.astype(np.float16)
         for k in ("x_l", "x_r", "h_l", "h_r", "c_l", "c_r")}

    W_i = np.asarray(inp["W_i"], f32)
    W_fl = np.asarray(inp["W_fl"], f32)
    W_fr = np.asarray(inp["W_fr"], f32)
    W_xin = np.asarray(inp["W_xin"], f32)
    W_o = np.asarray(inp["W_o"], f32)
    W_z = np.asarray(inp["W_z"], f32)
    W_g = np.asarray(inp["W_g"], f32)

    x_i = W_xin[0 * D:1 * D].T
    x_f = W_xin[1 * D:2 * D].T
    x_o = W_xin[2 * D:3 * D].T
    x_g = W_xin[3 * D:4 * D].T

    weights = {
        "wz": _prep_weight(np.ascontiguousarray(W_z.T)),
        "wi": _prep_weight(np.concatenate([W_i.T, x_i], axis=0)),
        "wfl": _prep_weight(np.ascontiguousarray(W_fl.T)),
        "wfr": _prep_weight(np.ascontiguousarray(W_fr.T)),
        "wg": _prep_weight(np.concatenate([W_g.T, x_g], axis=0)),
        "wo": _prep_weight(np.concatenate([W_o.T, x_o], axis=0)),
        "wxf": _prep_weight(x_f),
    }

    b = np.stack([np.asarray(inp[k], f32) for k in
                  ("b_z", "b_i", "b_fl", "b_fr", "b_g", "b_o")])
    bias = np.ascontiguousarray(b.reshape(6, MT, P).transpose(2, 0, 1))

    in_maps = []
    for c in range(N_CORES):
        cs = slice(c * NL, (c + 1) * NL)
        m = {
            "xb_l": np.ascontiguousarray(t["x_l"][:, cs]),
            "xb_r": np.ascontiguousarray(t["x_r"][:, cs]),
            "hb_l": np.ascontiguousarray(t["h_l"][:, cs]),
            "hb_r": np.ascontiguousarray(t["h_r"][:, cs]),
            "cb_l": np.ascontiguousarray(t["c_l"][:, cs]),
            "cb_r": np.ascontiguousarray(t["c_r"][:, cs]),
            "bias": bias,
        }
        m.update(weights)
        in_maps.append(m)
    return in_maps


def run(inputs, trace=False, trace_kwargs=None):
    from concourse.bass_utils import run_bass_kernel_spmd

    if trace:
        try:
            from hookfix import install_ntff_hook
            install_ntff_hook()
        except Exception:
            pass
    nc = _get_compiled()
    in_maps = _host_prep(inputs)
    res = run_bass_kernel_spmd(nc, in_maps, core_ids=list(range(N_CORES)),
                               trace=trace, **(trace_kwargs or {}))
    xT = np.concatenate([res.results[c]["xT_o"] for c in range(N_CORES)], axis=1)
    hT = np.concatenate([res.results[c]["hT_o"] for c in range(N_CORES)], axis=1)
    cT = np.concatenate([res.results[c]["cT_o"] for c in range(N_CORES)], axis=1)
    x_t = np.ascontiguousarray(xT.T.astype(np.float32))
    h_t = np.ascontiguousarray(hT.T.astype(np.float32))
    c_t = np.ascontiguousarray(cT.T.astype(np.float32))
    return (x_t, h_t, c_t), res


def kernel(**inputs):
    out, _ = run(inputs)
    return out
